# revision 57
# baseline (speedup 1.0000x reference)
"""Trainium2 Bass kernel for nn_NewtonLoss (segment_reduce).

Computes, for K refinement states over N atoms grouped into M molecules:
    sq[k,i]   = ||states_x[k,i,:] - x_target[i,:]||^2
    S[m,k]    = segment_sum(sq[k], molecule_id)
    per_state = sum_m valid_m * S[m,k]/c_m / V
    loss      = sum_k w_k * per_state_k        (w = normalized gamma powers)

Default strategy (variant "v4", 8-core SPMD, memory-bound):
  - The segment reduce collapses to a weighted dot product: per-atom
    weight w_i = 1/count(molecule(i)), so loss = sum_{k,i} w_k*w_i/V *
    ||st[k,i]-tgt[i]||^2 — order-independent, no scan, no contiguity.
  - Host stages each core's 250k-atom shard k-major fp16 [P, K, R*3]:
    every engine op is a long unit-stride [128, ~5880] row (DVE 2x mode,
    clean 11.76 KB DMA lines), vs. the older atom-major layouts whose
    3-element inner runs broke packing/alignment.
  - Weight-by-row-class: atoms are regrouped so each partition row's
    first S columns hold atoms of ONE molecule-size class; the row
    weight w_p is applied post-hoc to the tiny per-(rep,k) accumulator
    columns (one tensor_scalar_mul per group). Only the R-S "soup"
    columns carry explicit ratio weights sqrt(w_i/w_p) via one short
    DVE multiply. This cuts DVE element-touches from 2/elem to ~1.2.
  - Per rep per k: DMA st_k (all on the SP HWDGE queue — mixing queues
    measured slower), DVE sub, short DVE tail-mul, then the unweighted
    sum-of-squares reduce split by columns: [0:x1] ACT Square(accum_out),
    [x1:] DVE STT(accum_out) — per-ENGINE accum tiles + scratch pools so
    no tile has writers on two engines (cross-engine semaphore chains
    measured ~+6us). agrp reps share one accum tile per engine; outputs
    cycle through a fixed number of HBM blocks so the output size (and
    the per-call zeros upload) is O(1) in reps.
  - Host sums the 8 tiny per-core accumulators into the final scalar.

Host-side work on the big arrays is staging only (shard/layout/dtype
/permutation); all arithmetic on states/target runs on the NeuronCores.
Older variants kept: "scan"/"uw" (build_program), "uw16"/"uw8"
(build_program_v2), "v3" (k-major, per-atom sqrt(w) multiply).
"""

import os
import sys

import numpy as np

for _p in ("/opt/trn_rl_repo",):
    if os.path.isdir(_p) and _p not in sys.path:
        sys.path.insert(0, _p)

import concourse.bacc as bacc  # noqa: E402
import concourse.bass as bass  # noqa: E402
import concourse.tile as tile  # noqa: E402
from concourse import mybir  # noqa: E402

GAMMA = 0.7
NCORES = 8
P = 128  # partitions

# Full-problem geometry (N = 2_000_000 atoms):
#   per-core window = NTILES * P * R atoms; overlapping windows cover N.
K_FULL = 8
R_FULL = 128          # atoms per partition-row per tile
NTILES_FULL = 16
CHUNK_TILES_FULL = 4  # tiles per scan chunk

# v2 ("uw16") geometry: per-core shard = P * R2 atoms, fp16 host staging.
R2_FULL = 1960        # atoms per partition; SHARD2 = 250_880 >= 250_000
NCHUNKS2_FULL = 4

DEFAULT_VARIANT = "v4"


def build_program(K=K_FULL, ntiles=NTILES_FULL, R=R_FULL,
                  chunk_tiles=CHUNK_TILES_FULL, reps=1, variant="scan",
                  add1_engine="gpsimd", add2_engine="vector",
                  red_mode="act", stbufs=3, chbufs=2,
                  scan_engine="vector", mul_engine="vector",
                  stop_after="full", cast=True, dropmask=False,
                  num_devices=1):
    """Build the single-core Bass program (run SPMD on all cores).

    variant "scan": masked segmented scan + piece-end weights (W sparse).
    variant "uw":   per-atom 1/count weights, no scan (W dense).
    red_mode "ttr": fused multiply-reduce on DVE.
    red_mode "act": multiply on DVE, accumulate via ACT Copy(accum_out).
    """
    TILE = P * R
    SHARD = ntiles * TILE
    RD = R * 3
    nchunks = ntiles // chunk_tiles
    CH = chunk_tiles * R  # scan length per chunk per partition
    f32 = mybir.dt.float32
    f16 = mybir.dt.float16 if cast else f32
    add, mult = mybir.AluOpType.add, mybir.AluOpType.mult

    nc = bacc.Bacc("TRN2", target_bir_lowering=False, debug=False,
                   num_devices=num_devices)
    states = nc.dram_tensor("states", [K, SHARD, 3], f32, kind="ExternalInput").ap()
    target = nc.dram_tensor("target", [SHARD, 3], f32, kind="ExternalInput").ap()
    maskd = (None if dropmask else
             nc.dram_tensor("mask", [SHARD], f32, kind="ExternalInput").ap())
    wvecd = nc.dram_tensor("wvec", [SHARD], f32, kind="ExternalInput").ap()
    accd = nc.dram_tensor("acc", [P, nchunks * K], f32, kind="ExternalOutput").ap()

    # atom i lives at (tile t, partition p, row-pos r): i = t*TILE + p*R + r
    st_v = states.rearrange("k (t p r) d -> t p k (r d)", t=ntiles, p=P)
    tg_v = target.rearrange("(t p r) d -> p t (r d)", t=ntiles, p=P)
    mk_v = (None if dropmask else
            maskd.rearrange("(t p r) -> p t r", t=ntiles, p=P))
    wv_v = wvecd.rearrange("(t p r) -> p t r", t=ntiles, p=P)

    engines = {"vector": nc.vector, "gpsimd": nc.gpsimd}
    add1_e, add2_e = engines[add1_engine], engines[add2_engine]
    mul_e = engines[mul_engine]

    def scan_e(k):
        if scan_engine == "split":
            return nc.vector if k % 2 == 0 else nc.gpsimd
        return engines[scan_engine]

    with tile.TileContext(nc) as tc:
        with (
            tc.tile_pool(name="singles", bufs=1) as singles,
            tc.tile_pool(name="stp", bufs=stbufs) as stp,
            tc.tile_pool(name="dfp", bufs=2) as dfp,
            tc.tile_pool(name="sqp", bufs=2) as sqp,
            tc.tile_pool(name="tmpp", bufs=2) as tmpp,
            tc.tile_pool(name="chp", bufs=chbufs) as chp,
            tc.tile_pool(name="scp", bufs=2) as scp,
            tc.tile_pool(name="ttp", bufs=2) as ttp,
        ):
            tg_all = singles.tile([P, ntiles, RD], f16)
            (nc.gpsimd if cast else nc.sync).dma_start(out=tg_all, in_=tg_v)
            wv_all = singles.tile([P, ntiles, R], f32)
            nc.sync.dma_start(out=wv_all, in_=wv_v)
            if not dropmask:
                # load mask even if unused: a declared-but-stripped input
                # tensor crashes the pjrt exec path
                mk_all = singles.tile([P, ntiles, R], f32)
                nc.sync.dma_start(out=mk_all, in_=mk_v)
            acc = singles.tile([P, nchunks * K], f32)
            order = ["dma", "sub", "sq", "adds", "scan", "full"]
            lvl = order.index(stop_after)
            if lvl < 5:
                nc.vector.memset(acc, 0.0)

            for _rep in range(reps):
                for ch in range(nchunks):
                    sqbuf = chp.tile([P, K, chunk_tiles, R], f32)
                    for j in range(chunk_tiles):
                        t = ch * chunk_tiles + j
                        st = stp.tile([P, K, RD], f16)
                        (nc.gpsimd if cast else nc.sync).dma_start(
                            out=st, in_=st_v[t])
                        if lvl < 1:
                            continue
                        diff = dfp.tile([P, K, RD], f16)
                        tgs = tg_all[:, t, :]
                        tgb = bass.AP(
                            tensor=tgs.tensor, offset=tgs.offset,
                            ap=[list(tgs.ap[0]), [0, K], list(tgs.ap[-1])],
                        )
                        nc.vector.tensor_sub(diff, st, tgb)
                        if lvl < 2:
                            continue
                        sq = sqp.tile([P, K, RD], f32)
                        nc.scalar.square(sq, diff)
                        if lvl < 3:
                            continue
                        sq4 = sq.rearrange("p k (r d) -> p k r d", d=3)
                        tmp = tmpp.tile([P, K, R], f32)
                        add1_e.tensor_add(tmp, sq4[:, :, :, 0], sq4[:, :, :, 1])
                        add2_e.tensor_add(sqbuf[:, :, j, :], tmp, sq4[:, :, :, 2])
                    if lvl < 4:
                        continue
                    wv_ch = wv_all[:, ch * chunk_tiles:(ch + 1) * chunk_tiles, :]
                    wv_ch = wv_ch.rearrange("p t r -> p (t r)")
                    if variant == "scan":
                        mk_ch = mk_all[:, ch * chunk_tiles:(ch + 1) * chunk_tiles, :]
                        mk_ch = mk_ch.rearrange("p t r -> p (t r)")
                    for k in range(K):
                        red_in = sqbuf[:, k, :, :].rearrange("p t r -> p (t r)")
                        if variant == "scan":
                            scano = scp.tile([P, CH], f32)
                            scan_e(k).tensor_tensor_scan(
                                out=scano, data0=mk_ch, data1=red_in,
                                initial=0.0, op0=mult, op1=add)
                            red_in = scano
                        if lvl < 5:
                            continue
                        tto = ttp.tile([P, CH], f32)
                        acc_slot = acc[:, ch * K + k: ch * K + k + 1]
                        if red_mode == "stt":
                            mul_e.scalar_tensor_tensor(
                                out=tto, in0=red_in, scalar=1.0, in1=wv_ch,
                                op0=mult, op1=mult, accum_out=acc_slot)
                        elif red_mode == "act":
                            mul_e.tensor_mul(tto, red_in, wv_ch)
                            nc.scalar.activation(
                                tto, tto, mybir.ActivationFunctionType.Copy,
                                accum_out=acc_slot)
                        else:
                            nc.vector.tensor_tensor_reduce(
                                out=tto, in0=red_in, in1=wv_ch, scale=1.0,
                                scalar=0.0, op0=mult, op1=add,
                                accum_out=acc_slot)
            nc.sync.dma_start(out=accd, in_=acc)
    nc.compile()
    return nc


def build_program_v2(K=K_FULL, R=R2_FULL, nchunks=NCHUNKS2_FULL, reps=1,
                     pw=0, ga=8, stop="full", outq="scalar", accbufs=2,
                     stbufs=2, dfbufs=2, dwbufs=2, st8=False, dmaq="one",
                     whole=False, num_devices=1):
    """v2 single-core program (fp16, no scan): variant "uw16".

    loss contribution = sum_i w_i * ||st[k,i]-tgt[i]||^2 with per-atom
    weights w_i = 1/molecule_count (order-independent weighted dot).
    HBM layout (host-staged fp16): states [P, K, R*3], target [P, R*3],
    wvec3 [P, R*3] carrying sqrt(w) replicated x3.

    Per chunk (engine menu: DVE tensor_tensor runs 2x with fp16;
    scalar_tensor_tensor/reduce are 1x; ACT Square fuses square+accum):
      diff = st - tgt_broadcast      TT sub    (DVE; k<psub on GPSIMD)
      dw   = diff * sqw3_broadcast   TT mult   (k<pw on GPSIMD, rest DVE)
      per k: acc[ch,k] = sum(dw_k^2) ACT Square(accum_out) for k<ga,
                                     DVE STT(dw,dw,mult,accum) for k>=ga
    stop: "dma"|"sub"|"wmult"|"full" — truncate pipeline for stage benches.
    """
    assert R % nchunks == 0
    RC = R // nchunks
    R3 = R * 3
    K3 = K * 3
    f32, f16 = mybir.dt.float32, mybir.dt.float16
    mult, sub = mybir.AluOpType.mult, mybir.AluOpType.subtract
    lvl = ["dma", "sub", "wmult", "full"].index(stop)
    AGRP = 4  # reps per acc-output DMA group

    nc = bacc.Bacc("TRN2", target_bir_lowering=False, debug=False,
                   num_devices=num_devices)
    stdt = mybir.dt.float8e4 if st8 else f16
    # atom-major states layout: per partition, (r, k, d) contiguous ->
    # one long contiguous line per (partition, chunk) = few descriptors
    states = nc.dram_tensor("states", [P, R, K3], stdt, kind="ExternalInput").ap()
    target = nc.dram_tensor("target", [P, R3], f16, kind="ExternalInput").ap()
    wvec3 = nc.dram_tensor("wvec3", [P, R3], f16, kind="ExternalInput").ap()
    # per-rep output blocks: keep every rep's accums live (no DCE of the
    # repeat loop) at negligible DMA cost; combine() reads the last block
    accd = nc.dram_tensor("acc", [P, reps * nchunks * K], f32,
                          kind="ExternalOutput").ap()

    with tile.TileContext(nc) as tc:
        with (
            tc.tile_pool(name="singles", bufs=1) as singles,
            tc.tile_pool(name="stp", bufs=stbufs) as stp,
            tc.tile_pool(name="dfpp", bufs=dfbufs) as dfpp,
            tc.tile_pool(name="dfvp", bufs=dfbufs) as dfvp,
            tc.tile_pool(name="dwpp", bufs=dwbufs) as dwpp,
            tc.tile_pool(name="dwvp", bufs=dwbufs) as dwvp,
            tc.tile_pool(name="ttp", bufs=2) as ttp,
            tc.tile_pool(name="accp", bufs=2) as accp,
        ):
            tg_all = singles.tile([P, R3], f16)
            nc.sync.dma_start(out=tg_all, in_=target)
            wv_all = singles.tile([P, R3], f16)
            nc.sync.dma_start(out=wv_all, in_=wvec3)
            out_eng = {"sync": nc.sync, "scalar": nc.scalar,
                       "gpsimd": nc.gpsimd}[outq]

            def kdbcast(ap2d, r0, n):
                # [P, (r d)] row-major slice -> AP [P, (r:RC), (0,n), (d:3)]
                s = ap2d[:, r0 * 3:(r0 + RC) * 3]
                return bass.AP(tensor=s.tensor, offset=s.offset,
                               ap=[list(s.ap[0]), [3, RC], [0, n], [1, 3]])

            acc = None
            for _rep in range(reps):
                if _rep % AGRP == 0:
                    na = min(AGRP, reps - _rep)
                    acc = accp.tile([P, na, nchunks * K], f32)
                if whole:
                    # one DMA instruction per rep for the whole shard:
                    # probes/avoids per-DMA-instruction fixed costs.
                    # cap per-descriptor payload below the 64 KB SDMA limit
                    stw = stp.tile([P, R, K3], f16)
                    (nc.gpsimd if st8 else nc.sync).dma_start(
                        out=stw, in_=states, max_dma_last_dim=16384)
                for ch in range(nchunks):
                    r0 = ch * RC
                    if whole:
                        st = stw[:, r0:r0 + RC, :]
                    else:
                        st = stp.tile([P, RC, K3], f16)
                        # fp8 HBM -> fp16 SBUF casts go through the gpsimd
                        # SWDGE; dmaq="alt" stripes over both HWDGE queues
                        if st8:
                            steng = nc.gpsimd
                        elif dmaq == "alt":
                            steng = nc.sync if ch % 2 == 0 else nc.scalar
                        else:
                            steng = nc.sync
                        steng.dma_start(out=st, in_=states[:, r0:r0 + RC, :])
                    if lvl < 1:
                        continue

                    st4 = st.rearrange("p r (k d) -> p r k d", d=3)
                    # pool-owned k range [0, pw): sub+wmult on GPSIMD into
                    # its own tiles so ACT accums for those k don't wait on
                    # the DVE chain (and vice versa)
                    pg = pw
                    if pg > 0:
                        dfP = dfpp.tile([P, RC, pg, 3], f16)
                        dwP = dwpp.tile([P, RC, pg, 3], f16)
                        nc.gpsimd.tensor_sub(dfP, st4[:, :, :pg, :],
                                             kdbcast(tg_all, r0, pg))
                        nc.gpsimd.tensor_mul(dwP, dfP,
                                             kdbcast(wv_all, r0, pg))
                    if pg < K:
                        dfV = dfvp.tile([P, RC, K - pg, 3], f16)
                        dwV = dwvp.tile([P, RC, K - pg, 3], f16)
                        nc.vector.tensor_sub(dfV, st4[:, :, pg:, :],
                                             kdbcast(tg_all, r0, K - pg))
                        if lvl < 2:
                            continue
                        nc.vector.tensor_mul(dwV, dfV,
                                             kdbcast(wv_all, r0, K - pg))
                    if lvl < 3:
                        continue

                    for k in range(K):
                        tto = ttp.tile([P, RC, 3], f16)
                        slot = acc[:, _rep % AGRP,
                                   ch * K + k: ch * K + k + 1]
                        dwk = (dwP[:, :, k, :] if k < pg
                               else dwV[:, :, k - pg, :])
                        if k < ga:
                            nc.scalar.activation(
                                tto, dwk,
                                mybir.ActivationFunctionType.Square,
                                accum_out=slot)
                        else:
                            nc.vector.scalar_tensor_tensor(
                                out=tto, in0=dwk, scalar=1.0,
                                in1=dwk, op0=mult, op1=mult,
                                accum_out=slot)
                if _rep % AGRP == AGRP - 1 or _rep == reps - 1:
                    g0 = (_rep // AGRP) * AGRP
                    na = _rep - g0 + 1
                    o0 = g0 * nchunks * K
                    out_eng.dma_start(
                        out=accd[:, o0:o0 + na * nchunks * K],
                        in_=acc[:, :na].rearrange("p a c -> p (a c)"))
    nc.compile()
    return nc


def build_program_v3(K=K_FULL, R=R2_FULL, reps=1, gp_pairs=(), gp_red=(),
                     dve_red=(), dmaq="alt", agrp=4, stbufs=3, dfbufs=3,
                     dwbufs=2, stop="full", kchunk=1, maxdesc=0,
                     st8=True, subcast=False, nodma=False, oblk=8,
                     num_devices=1):
    """v3 single-core program: k-major fp16 layout, long contiguous ops.

    HBM layout (host-staged): states [P, K, R*3] fp16 (per partition, each
    k's R*3 coords contiguous), target [P, R*3] fp16, wvec3 [P, R*3] fp16
    (sqrt(1/molecule_count) per atom, replicated x3).

    Per rep, per k (all ops full [P, R*3] unit-stride fp16 => DVE 2x mode,
    ACT single fused square+accumulate, DMA lines of R*3*2 bytes):
      DMA   st_k  <- states[:, k, :]          (alternate sync/scalar HWDGE)
      sub   diff = st_k - target              (DVE, or GPSIMD for k in
      mul   dw   = diff * wvec3                gp_pairs to offload DVE)
      red   acc[k] = sum(dw^2)                (ACT Square(accum_out); k in
                                               dve_red/gp_red use STT there)
    Every agrp reps the [P, K] accumulators DMA to a distinct HBM block
    (keeps the repeat loop live for honest marginal timing).
    """
    R3 = R * 3
    f32, f16 = mybir.dt.float32, mybir.dt.float16
    stdt = mybir.dt.float8e4 if st8 else f16
    mult = mybir.AluOpType.mult

    nc = bacc.Bacc("TRN2", target_bir_lowering=False, debug=False,
                   num_devices=num_devices)
    states = nc.dram_tensor("states", [P, K, R3], stdt, kind="ExternalInput").ap()
    target = nc.dram_tensor("target", [P, R3], f16, kind="ExternalInput").ap()
    wvec3 = nc.dram_tensor("wvec3", [P, R3], f16, kind="ExternalInput").ap()
    # out blocks cycle mod oblk: per-rep out-DMAs stay live but the output
    # (and the per-run zeros upload) stays O(oblk), not O(reps)
    oblk = min(oblk, ((reps + agrp - 1) // agrp) * agrp)
    oblk = max(oblk, agrp)
    oblk = ((oblk + agrp - 1) // agrp) * agrp
    accd = nc.dram_tensor("acc", [P, oblk * K], f32, kind="ExternalOutput").ap()

    with tile.TileContext(nc) as tc:
        with (
            tc.tile_pool(name="singles", bufs=1) as singles,
            tc.tile_pool(name="stp", bufs=stbufs) as stp,
            tc.tile_pool(name="dfp", bufs=dfbufs) as dfp,
            tc.tile_pool(name="dwp", bufs=dwbufs) as dwp,
            tc.tile_pool(name="accp", bufs=2) as accp,
        ):
            tg_all = singles.tile([P, R3], f16)
            nc.sync.dma_start(out=tg_all, in_=target)
            wv_all = singles.tile([P, R3], f16)
            nc.sync.dma_start(out=wv_all, in_=wvec3)
            lvl = ["dma", "sub", "mul", "full"].index(stop)

            queues = {"alt": (nc.sync, nc.scalar),
                      "sync": (nc.sync,),
                      "scalar": (nc.scalar,),
                      "tri": (nc.sync, nc.scalar, nc.gpsimd)}[dmaq]
            dmakw = {"max_dma_last_dim": maxdesc} if maxdesc else {}

            stfix = None
            if nodma:
                # compute-only probe: one static states tile, no streaming
                stfix = singles.tile([P, kchunk, R3], stdt)
                nc.sync.dma_start(out=stfix, in_=states[:, :kchunk, :])

            acc = None
            for _rep in range(reps):
                if _rep % agrp == 0:
                    na = min(agrp, reps - _rep)
                    acc = accp.tile([P, na, K], f32)
                    if lvl < 3:
                        nc.vector.memset(acc, 0.0)
                for kc in range(K // kchunk):
                    k0 = kc * kchunk
                    if nodma:
                        stc = stfix
                    else:
                        stc = stp.tile([P, kchunk, R3], stdt)
                        dq = queues[kc % len(queues)]
                        dq.dma_start(out=stc, in_=states[:, k0:k0 + kchunk, :],
                                     **dmakw)
                    if lvl < 1:
                        continue
                    for j in range(kchunk):
                        k = k0 + j
                        st = stc[:, j, :]
                        eng = nc.gpsimd if k in gp_pairs else nc.vector
                        if st8 and subcast:
                            st16 = dfp.tile([P, R3], f16)
                            eng.tensor_copy(st16, st)
                            st = st16
                        diff = dfp.tile([P, R3], f16)
                        eng.tensor_sub(diff, st, tg_all)
                        if lvl < 2:
                            continue
                        dw = dwp.tile([P, R3], f16)
                        eng.tensor_mul(dw, diff, wv_all)
                        if lvl < 3:
                            continue
                        slot = acc[:, _rep % agrp, k:k + 1]
                        acto = dfp.tile([P, R3], f16)
                        if k in gp_red:
                            nc.gpsimd.scalar_tensor_tensor(
                                out=acto, in0=dw, scalar=1.0, in1=dw,
                                op0=mult, op1=mult, accum_out=slot)
                        elif k in dve_red:
                            nc.vector.scalar_tensor_tensor(
                                out=acto, in0=dw, scalar=1.0, in1=dw,
                                op0=mult, op1=mult, accum_out=slot)
                        else:
                            nc.scalar.activation(
                                acto, dw, mybir.ActivationFunctionType.Square,
                                accum_out=slot)
                if _rep % agrp == agrp - 1 or _rep == reps - 1:
                    g0 = (_rep // agrp) * agrp
                    na = _rep - g0 + 1
                    o0 = (g0 % oblk) * K
                    nc.sync.dma_start(
                        out=accd[:, o0:o0 + na * K],
                        in_=acc[:, :na].rearrange("p a c -> p (a c)"))
    nc.compile()
    return nc


V4_S = 1568  # pure-class head columns per row (tail = R - S soup columns)
V4_X1, V4_X2 = 5150, 5880  # default reduce segment boundaries (elements)
V4_AGRP = 16  # accumulation group size (reps per acc tile / out-DMA)


def v4_segs(x1, x2, R3):
    """Reduce column segments: [0:x1] ACT, [x1:x2] DVE, [x2:R3] GPSIMD."""
    segs = []
    if x1 > 0:
        segs.append((0, min(x1, R3), "act"))
    if x2 > x1 and x1 < R3:
        segs.append((x1, min(x2, R3), "dve"))
    if x2 < R3:
        segs.append((x2, R3, "gp"))
    return segs


def build_program_v4(K=K_FULL, R=R2_FULL, S=V4_S, reps=1, x1=V4_X1, x2=V4_X2,
                     dred="stt", dmaq="sync", agrp=V4_AGRP, stbufs=4, dfbufs=4,
                     oblk=8, stop="full", tmeng="dve", accps=False,
                     outq="scalar", num_devices=1):
    """v4: weight-by-row-class. Atoms are regrouped on host so that each
    partition row's first S columns hold atoms of a single molecule-size
    class (shared weight w_p); the remaining R-S "soup" columns carry
    explicit ratio weights sqrt(w_i/w_p) applied by one short DVE multiply.
    Row weights w_p are applied POST-HOC to the tiny per-(rep,k) accum
    columns (one tensor_scalar_mul per agrp group), so the per-element
    reduce is a plain unweighted sum of squares and can be split freely
    across ACT (Square) / DVE (TTR or STT) / GPSIMD by column ranges.

    Per rep, per k (layout as v3: [P, K, R*3] fp16 k-major):
      DMA   st_k <- states[:, k, :]
      sub   diff = st_k - target                  (DVE, full row)
      mul   diff[:, S3:] *= wtail3                (DVE, tail only, in-place)
      red   per segment [lo:hi): acc slot = sum(diff^2)  (ACT/DVE/GP)
    """
    R3, S3 = R * 3, S * 3
    T3 = R3 - S3
    x1 = min(x1, R3) & ~1
    x2 = min(max(x2, x1), R3) & ~1
    segs = v4_segs(x1, x2, R3)
    nsl = len(segs)  # accum slots per k
    f32, f16 = mybir.dt.float32, mybir.dt.float16
    mult, add = mybir.AluOpType.mult, mybir.AluOpType.add

    nc = bacc.Bacc("TRN2", target_bir_lowering=False, debug=False,
                   num_devices=num_devices)
    states = nc.dram_tensor("states", [P, K, R3], f16, kind="ExternalInput").ap()
    target = nc.dram_tensor("target", [P, R3], f16, kind="ExternalInput").ap()
    wtail3 = nc.dram_tensor("wtail3", [P, T3], f16, kind="ExternalInput").ap()
    scalesd = nc.dram_tensor("scales", [P, 2], f32, kind="ExternalInput").ap()
    oblk = min(oblk, ((reps + agrp - 1) // agrp) * agrp)
    oblk = max(oblk, agrp)
    oblk = ((oblk + agrp - 1) // agrp) * agrp
    G = oblk // agrp  # cycling output group-blocks
    # output layout: [P, G, nsl, agrp*K]: per group-block, one contiguous
    # [agrp*K] range per segment (single-engine accum tiles DMA whole)
    accd = nc.dram_tensor("acc", [P, G * nsl * agrp * K], f32,
                          kind="ExternalOutput").ap()
    lvl = ["dma", "sub", "mul", "full"].index(stop)

    with tile.TileContext(nc) as tc:
        with (
            tc.tile_pool(name="singles", bufs=1) as singles,
            tc.tile_pool(name="stp", bufs=stbufs) as stp,
            tc.tile_pool(name="dfp", bufs=dfbufs) as dfp,
            tc.tile_pool(name="ao0", bufs=2) as ao0,
            tc.tile_pool(name="ao1", bufs=2) as ao1,
            tc.tile_pool(name="ac0", bufs=2) as ac0,
            tc.tile_pool(name="ac1", bufs=2) as ac1,
        ):
            tg_all = singles.tile([P, R3], f16)
            nc.sync.dma_start(out=tg_all, in_=target)
            wt_all = singles.tile([P, T3], f16)
            nc.sync.dma_start(out=wt_all, in_=wtail3)
            scales = singles.tile([P, 2], f32)
            nc.sync.dma_start(out=scales, in_=scalesd)
            sw_ap, w_ap = scales[:, 0:1], scales[:, 1:2]
            actp = [ao0, ao1]   # per-segment scratch pools (single engine)
            accp = [ac0, ac1]   # per-segment accum pools (single engine)
            assert nsl <= 2

            accs = [None] * nsl
            for _rep in range(reps):
                if _rep % agrp == 0:
                    na = min(agrp, reps - _rep)
                    for j in range(nsl):
                        accs[j] = accp[j].tile(
                            [P, na, K], f32, name=f"acc{j}",
                            space="PSUM" if accps else "SBUF")
                        if lvl < 3:
                            nc.vector.memset(accs[j], 0.0)
                for k in range(K):
                    st = stp.tile([P, R3], f16)
                    if dmaq == "alt":
                        dq = nc.sync if k % 2 == 0 else nc.scalar
                    elif dmaq == "half":
                        dq = nc.sync if k < K // 2 else nc.scalar
                    elif dmaq == "scalar":
                        dq = nc.scalar
                    else:
                        dq = nc.sync
                    dq.dma_start(out=st, in_=states[:, k, :])
                    if lvl < 1:
                        continue
                    diff = dfp.tile([P, R3], f16)
                    nc.vector.tensor_sub(diff, st, tg_all)
                    if lvl < 2:
                        continue
                    tme = nc.gpsimd if tmeng == "gp" else nc.vector
                    tme.tensor_mul(diff[:, S3:], diff[:, S3:], wt_all)
                    if lvl < 3:
                        continue
                    ar = _rep % agrp
                    for j, (lo_, hi_, seng) in enumerate(segs):
                        slot = accs[j][:, ar, k:k + 1]
                        d_ = diff[:, lo_:hi_]
                        o_ = actp[j].tile([P, hi_ - lo_], f16)
                        if seng == "act":
                            nc.scalar.activation(
                                o_, d_, mybir.ActivationFunctionType.Square,
                                accum_out=slot)
                        else:
                            eng = nc.vector if seng == "dve" else nc.gpsimd
                            if dred == "ttr" and seng == "dve":
                                eng.tensor_tensor_reduce(
                                    out=o_, in0=d_, in1=d_, scale=1.0,
                                    scalar=0.0, op0=mult, op1=add,
                                    accum_out=slot)
                            else:
                                eng.scalar_tensor_tensor(
                                    out=o_, in0=d_, scalar=1.0, in1=d_,
                                    op0=mult, op1=mult, accum_out=slot)
                if _rep % agrp == agrp - 1 or _rep == reps - 1:
                    g0 = (_rep // agrp) * agrp
                    na = _rep - g0 + 1
                    gi = (g0 // agrp) % G
                    for j in range(nsl):
                        accf = accs[j][:, :na].rearrange("p a c -> p (a c)")
                        # post-hoc per-row weight: acc *= w_p (tiny op)
                        if accps:  # PSUM accs: scale into SBUF for the DMA
                            acco = actp[j].tile([P, na * K], f32,
                                                name=f"acco{j}")
                            nc.vector.tensor_scalar_mul(acco, accf, w_ap)
                            accf = acco
                        else:
                            nc.vector.tensor_scalar_mul(accf, accf, w_ap)
                        o0 = (gi * nsl + j) * agrp * K
                        oeng = nc.scalar if outq == "scalar" else nc.sync
                        oeng.dma_start(out=accd[:, o0:o0 + na * K],
                                       in_=accf)
    nc.compile()
    return nc


def host_prep_v4(states_x, x_target, molecule_id, num_molecules,
                 ncores=NCORES, K=K_FULL, R=R2_FULL, S=V4_S):
    """Regroup atoms by molecule-size class into [P, R] rows.

    Row p: columns [0:S) hold S atoms of one class (count v_p) -> weight
    applied via per-partition scale; columns [S:R) hold leftover "soup"
    atoms with explicit ratio weights sqrt(v_p / c_i) (0 on padding).
    """
    R3 = R * 3
    T = R - S
    N = molecule_id.shape[0]
    M = int(num_molecules)
    assert N % ncores == 0
    OWN = N // ncores

    ids = np.asarray(molecule_id).astype(np.int64)
    counts = np.bincount(ids, minlength=M)
    V = int((counts > 0).sum())
    c_atom = counts[ids]  # per-atom molecule size (count >= 1)

    states_x = np.asarray(states_x)
    x_target = np.asarray(x_target)

    in_maps = []
    for c in range(ncores):
        lo = c * OWN
        ca = c_atom[lo:lo + OWN]
        order = np.argsort(ca, kind="stable")  # group same-count atoms
        cs = ca[order]
        # pure chunks of S equal-count atoms
        chunks = []  # (count_value, idx_array)
        soup_idx = []
        start = 0
        for v in np.unique(cs):
            seg = order[start:start + int((cs == v).sum())]
            start += seg.size
            nch = seg.size // S
            for j in range(nch):
                chunks.append((int(v), seg[j * S:(j + 1) * S]))
            soup_idx.append(seg[nch * S:])
        assert len(chunks) >= P, (c, len(chunks))
        for v, seg in chunks[P:]:
            soup_idx.append(seg)
        chunks = chunks[:P]
        soup = np.concatenate(soup_idx) if soup_idx else np.empty(0, np.int64)
        assert soup.size <= P * T, (soup.size, P * T)

        # per-row global atom indices; padding uses atom 0 with ratio 0
        perm = np.zeros((P, R), np.int64)
        vrow = np.zeros(P, np.float64)
        wt = np.zeros((P, T), np.float32)
        for p, (v, seg) in enumerate(chunks):
            perm[p, :S] = lo + seg
            vrow[p] = v
        nsp = soup.size
        soup_rows = np.zeros((P, T), np.int64)
        soup_mask = np.zeros((P, T), bool)
        flat = np.arange(nsp)
        soup_rows.reshape(-1)[:nsp] = lo + soup
        soup_mask.reshape(-1)[:nsp] = True
        perm[:, S:] = soup_rows
        # ratio weights sqrt(v_row / c_soup); 0 for padding
        cs_soup = c_atom[soup_rows]  # (P, T) counts (garbage on pad)
        wt = np.where(soup_mask,
                      np.sqrt(vrow[:, None] / np.maximum(cs_soup, 1)),
                      0.0).astype(np.float32)

        pf = perm.reshape(-1)
        st = states_x[:, pf, :].astype(np.float16)  # (K, P*R, 3)
        st = st.reshape(K, P, R, 3).transpose(1, 0, 2, 3).reshape(P, K, R3)
        tg = x_target[pf].astype(np.float16).reshape(P, R3)
        wt3 = np.repeat(wt, 3, axis=1).astype(np.float16)  # (P, T*3)
        scales = np.stack([np.sqrt(1.0 / vrow), 1.0 / vrow],
                          axis=1).astype(np.float32)  # (P, 2)

        in_maps.append({
            "states": np.ascontiguousarray(st),
            "target": np.ascontiguousarray(tg),
            "wtail3": np.ascontiguousarray(wt3),
            "scales": np.ascontiguousarray(scales),
        })
    return in_maps, V


def host_prep_v3(states_x, x_target, molecule_id, num_molecules,
                 ncores=NCORES, K=K_FULL, R=R2_FULL, st8=True):
    """Shard + stage inputs in k-major [P, K, R*3] layout.

    states fp8e4m3 (st8) or fp16; target/weights fp16."""
    import ml_dtypes
    st_dt = ml_dtypes.float8_e4m3 if st8 else np.float16
    R3 = R * 3
    N = molecule_id.shape[0]
    M = int(num_molecules)
    assert N % ncores == 0
    OWN = N // ncores
    assert P * R >= OWN

    ids = np.asarray(molecule_id).astype(np.int64)
    counts = np.bincount(ids, minlength=M)
    V = int((counts > 0).sum())
    inv_c = np.zeros(M, np.float32)
    nz = counts > 0
    inv_c[nz] = 1.0 / counts[nz]
    w_full = inv_c[ids]  # (N,) fp32

    states_x = np.asarray(states_x)
    x_target = np.asarray(x_target)

    nfull = OWN // R            # full partitions per shard
    rem = OWN - nfull * R       # atoms in the last (padded) partition

    in_maps = []
    for c in range(ncores):
        lo, hi = c * OWN, (c + 1) * OWN
        mid = lo + nfull * R

        st = np.zeros((P, K, R, 3), st_dt)
        src = states_x[:, lo:mid, :].astype(st_dt)
        st[:nfull] = src.reshape(K, nfull, R, 3).transpose(1, 0, 2, 3)
        if rem:
            st[nfull, :, :rem] = states_x[:, mid:hi, :].astype(st_dt)

        tg = np.zeros((P, R, 3), np.float16)
        tg[:nfull] = x_target[lo:mid].astype(np.float16).reshape(nfull, R, 3)
        if rem:
            tg[nfull, :rem, :] = x_target[mid:hi]

        w = np.zeros((P, R), np.float32)
        w[:nfull] = w_full[lo:mid].reshape(nfull, R)
        if rem:
            w[nfull, :rem] = w_full[mid:hi]
        # stage sqrt(w): device computes sum((diff*sqrt_w)^2) = sum(w*diff^2)
        w3 = np.broadcast_to(np.sqrt(w)[:, :, None], (P, R, 3)).astype(np.float16)

        in_maps.append({
            "states": st.reshape(P, K, R3),
            "target": np.ascontiguousarray(tg.reshape(P, R3)),
            "wvec3": np.ascontiguousarray(w3.reshape(P, R3)),
        })
    return in_maps, V


def host_prep_v2(states_x, x_target, molecule_id, num_molecules,
                 ncores=NCORES, K=K_FULL, R=R2_FULL, st8=False):
    """Shard + stage inputs as fp16 (states optionally fp8e4m3) in
    [P, K, R*3] / [P, R*3] layout."""
    import ml_dtypes
    st_dt = ml_dtypes.float8_e4m3 if st8 else np.float16
    SHARD = P * R
    N = molecule_id.shape[0]
    M = int(num_molecules)
    assert N % ncores == 0
    OWN = N // ncores
    assert SHARD >= OWN

    ids = np.asarray(molecule_id).astype(np.int64)
    counts = np.bincount(ids, minlength=M)
    V = int((counts > 0).sum())
    inv_c = np.zeros(M, np.float32)
    nz = counts > 0
    inv_c[nz] = 1.0 / counts[nz]
    w_full = inv_c[ids]  # (N,) fp32

    states_x = np.asarray(states_x)
    x_target = np.asarray(x_target)

    nfull = OWN // R            # full partitions per shard
    rem = OWN - nfull * R       # atoms in the last (padded) partition

    in_maps = []
    for c in range(ncores):
        lo, hi = c * OWN, (c + 1) * OWN
        mid = lo + nfull * R

        # atom-major: [P, R, K, 3] so each (partition, chunk) DMA line is
        # one long contiguous run (minimal descriptor count)
        st = np.zeros((P, R, K, 3), st_dt)
        src = states_x[:, lo:mid, :].astype(st_dt)
        st[:nfull] = src.reshape(K, nfull, R, 3).transpose(1, 2, 0, 3)
        if rem:
            st[nfull, :rem] = states_x[:, mid:hi, :].astype(st_dt).transpose(1, 0, 2)

        tg = np.zeros((P, R, 3), np.float16)
        tg[:nfull] = x_target[lo:mid].astype(np.float16).reshape(nfull, R, 3)
        if rem:
            tg[nfull, :rem, :] = x_target[mid:hi]

        w = np.zeros((P, R), np.float32)
        w[:nfull] = w_full[lo:mid].reshape(nfull, R)
        if rem:
            w[nfull, :rem] = w_full[mid:hi]
        # stage sqrt(w): device computes sum((diff*sqrt_w)^2) = sum(w*diff^2)
        w3 = np.broadcast_to(np.sqrt(w)[:, :, None], (P, R, 3)).astype(np.float16)

        in_maps.append({
            "states": st.reshape(P, R, K * 3),
            "target": np.ascontiguousarray(tg.reshape(P, R * 3)),
            "wvec3": np.ascontiguousarray(w3.reshape(P, R * 3)),
        })
    return in_maps, V


def host_prep(states_x, x_target, molecule_id, num_molecules,
              ncores=NCORES, K=K_FULL, ntiles=NTILES_FULL, R=R_FULL,
              variant=DEFAULT_VARIANT):
    if variant == "v4":
        return host_prep_v4(states_x, x_target, molecule_id, num_molecules,
                            ncores=ncores, K=K)
    if variant == "v3":
        return host_prep_v3(states_x, x_target, molecule_id, num_molecules,
                            ncores=ncores, K=K)
    if variant in ("uw16", "uw8"):
        return host_prep_v2(states_x, x_target, molecule_id, num_molecules,
                            ncores=ncores, K=K, st8=(variant == "uw8"))
    return _host_prep_v1(states_x, x_target, molecule_id, num_molecules,
                         ncores=ncores, K=K, ntiles=ntiles, R=R,
                         variant=variant)


def _host_prep_v1(states_x, x_target, molecule_id, num_molecules,
                  ncores=NCORES, K=K_FULL, ntiles=NTILES_FULL, R=R_FULL,
                  variant="scan"):
    """Shard inputs into per-core windows; build mask/weight vectors.

    Returns (in_maps, V) where in_maps[c] are the named inputs for core c.
    """
    TILE = P * R
    SHARD = ntiles * TILE
    N = molecule_id.shape[0]
    M = int(num_molecules)
    assert N % ncores == 0
    OWN = N // ncores
    assert SHARD >= OWN, (SHARD, OWN)

    ids = np.asarray(molecule_id).astype(np.int64)
    counts = np.bincount(ids, minlength=M)
    V = int((counts > 0).sum())
    inv_c = np.zeros(M, np.float64)
    nz = counts > 0
    inv_c[nz] = 1.0 / counts[nz]

    states_x = np.asarray(states_x)
    x_target = np.asarray(x_target)

    r_idx = np.arange(SHARD, dtype=np.int64) % R

    in_maps = []
    for c in range(ncores):
        S_c = 0 if ncores == 1 else (c * (N - SHARD)) // (ncores - 1)
        own_lo, own_hi = c * OWN - S_c, (c + 1) * OWN - S_c
        assert own_lo >= 0 and own_hi <= SHARD

        idw = ids[S_c:S_c + SHARD]
        pos = np.arange(SHARD, dtype=np.int64)
        owned = (pos >= own_lo) & (pos < own_hi)

        if variant == "uw":
            m = np.zeros(SHARD, np.float32)
            w = np.where(owned, inv_c[idw], 0.0)
        else:
            same_prev = np.zeros(SHARD, bool)
            same_prev[1:] = idw[1:] == idw[:-1]
            m = (r_idx > 0) & same_prev & owned
            m[1:] &= owned[:-1]

            nxt_same = np.zeros(SHARD, bool)
            nxt_same[:-1] = idw[:-1] == idw[1:]
            nxt_same[:-1] &= owned[1:]
            nxt_same &= r_idx < (R - 1)
            w = np.where(owned & ~nxt_same, inv_c[idw], 0.0)

        in_maps.append({
            "states": np.ascontiguousarray(states_x[:, S_c:S_c + SHARD, :],
                                           dtype=np.float32),
            "target": np.ascontiguousarray(x_target[S_c:S_c + SHARD, :],
                                           dtype=np.float32),
            "mask": np.asarray(m, np.float32),
            "wvec": np.asarray(w, np.float32),
        })
    return in_maps, V


def combine_v4(results, V, K=K_FULL, nsl=2, agrp=None):
    agrp = V4_AGRP if agrp is None else agrp
    """v4 output [P, G, nsl, agrp*K]: rep 0's slots = first K columns of
    each segment block in group-block 0 (always written)."""
    total = np.zeros(K, np.float64)
    for r in results:
        acc = np.asarray(r["acc"]).astype(np.float64)
        for j in range(nsl):
            total += acc[:, j * agrp * K:j * agrp * K + K].sum(axis=0)
    per_state = total / V
    w = GAMMA ** ((K - 1) - np.arange(K, dtype=np.float64))
    w = w / w.sum()
    return np.float32((w * per_state).sum())


def combine(results, V, K=K_FULL, nchunks=NCHUNKS2_FULL):
    """Sum per-core accumulators into the final scalar loss.

    acc is [P, reps*nchunks*K]; only the last rep's nchunks*K block is
    read (every rep computes the same sums). v3 passes nchunks=1.
    """
    total = np.zeros(K, np.float64)
    for r in results:
        acc = np.asarray(r["acc"]).astype(np.float64)
        if nchunks == 1:
            acc = acc[:, :K]  # v3: block 0 is always rep 0's output
        elif nchunks < 0:  # v4: -nchunks interleaved slots per k in block 0
            nsl = -nchunks
            acc = acc[:, :nsl * K].reshape(P, K, nsl).sum(axis=2)
        elif nchunks * K < acc.shape[-1]:  # [P, reps*nchunks*K]
            acc = acc[:, -nchunks * K:]  # last rep's blocks
        total += acc.reshape(P, -1, K).sum(axis=(0, 1))
    per_state = total / V
    w = GAMMA ** ((K - 1) - np.arange(K, dtype=np.float64))
    w = w / w.sum()
    return np.float32((w * per_state).sum())


class Runner:
    """Caches the compiled PJRT executable for repeated SPMD runs."""

    def __init__(self, nc, n_cores=NCORES, n_inner=1):
        import jax
        from jax.experimental.shard_map import shard_map
        from jax.sharding import Mesh, PartitionSpec
        from concourse import bass2jax

        bass2jax.install_neuronx_cc_hook()
        self.jax = jax
        self.nc = nc
        self.n_cores = n_cores

        partition_name = (nc.partition_id_tensor.name
                          if nc.partition_id_tensor else None)
        in_names, out_names, out_avals, zero_outs = [], [], [], []
        for alloc in nc.m.functions[0].allocations:
            if not isinstance(alloc, mybir.MemoryLocationSet):
                continue
            name = alloc.memorylocations[0].name
            if alloc.kind == "ExternalInput":
                if name != partition_name:
                    in_names.append(name)
            elif alloc.kind == "ExternalOutput":
                shape = tuple(alloc.tensor_shape)
                dtype = mybir.dt.np(alloc.dtype)
                out_names.append(name)
                out_avals.append(jax.core.ShapedArray(shape, dtype))
                zero_outs.append(np.zeros(shape, dtype))
        self.in_names, self.out_names = in_names, out_names
        self.out_avals, self.zero_outs = out_avals, zero_outs
        n_params = len(in_names)
        all_in_names = list(in_names) + list(out_names)
        if partition_name is not None:
            all_in_names.append(partition_name)

        def _body(*args):
            ins = list(args[:n_params])
            cur_zeros = list(args[n_params:n_params + len(out_names)])
            extra = ([bass2jax.partition_id_tensor()]
                     if partition_name is not None else [])
            outs = tuple(cur_zeros)
            for _ in range(n_inner):
                # chain outputs into the next call's output buffers: keeps
                # every invocation live (no CSE/DCE) and is a no-op since
                # the kernel fully overwrites its outputs
                outs = bass2jax._bass_exec_p.bind(
                    *ins, *outs, *extra,
                    out_avals=tuple(out_avals),
                    in_names=tuple(all_in_names),
                    out_names=tuple(out_names),
                    lowering_input_output_aliases=(),
                    sim_require_finite=True,
                    sim_require_nnan=True,
                    nc=nc,
                )
            return tuple(outs)

        devices = jax.devices()[:n_cores]
        assert len(devices) == n_cores
        self.mesh = Mesh(np.asarray(devices), ("core",))
        self.pspec = PartitionSpec("core")
        n_outs = len(out_names)
        in_specs = (self.pspec,) * (n_params + n_outs)
        out_specs = (self.pspec,) * n_outs
        # no donation: the zero out-buffers are device_put once and reused
        # across calls, so per-call host->device upload stays O(1)
        self.fn = jax.jit(
            shard_map(_body, mesh=self.mesh, in_specs=in_specs,
                      out_specs=out_specs, check_rep=False),
            keep_unused=True)
        self._dev_zeros = None

    def concat_inputs(self, in_maps):
        return [np.concatenate([np.asarray(in_maps[c][n])
                                for c in range(self.n_cores)], axis=0)
                for n in self.in_names]

    def device_put(self, concat_in):
        from jax.sharding import NamedSharding
        sh = NamedSharding(self.mesh, self.pspec)
        return [self.jax.device_put(a, sh) for a in concat_in]

    def run_dev(self, dev_args):
        if self._dev_zeros is None:
            from jax.sharding import NamedSharding
            sh = NamedSharding(self.mesh, self.pspec)
            self._dev_zeros = [
                self.jax.device_put(
                    np.zeros((self.n_cores * z.shape[0], *z.shape[1:]),
                             z.dtype), sh)
                for z in self.zero_outs]
        out = self.fn(*dev_args, *self._dev_zeros)
        return self.jax.block_until_ready(out)

    def run(self, in_maps):
        out_arrs = self.run_dev(self.device_put(self.concat_inputs(in_maps)))
        return [
            {n: np.asarray(out_arrs[i]).reshape(
                self.n_cores, *self.out_avals[i].shape)[c]
             for i, n in enumerate(self.out_names)}
            for c in range(self.n_cores)
        ]


_CACHE = {}


def get_runner(variant=DEFAULT_VARIANT, reps=1, n_inner=1, **kw):
    key = (variant, reps, n_inner, tuple(sorted(kw.items())))
    if key not in _CACHE:
        if variant == "v4":
            nc = build_program_v4(reps=reps, **kw)
        elif variant == "v3":
            nc = build_program_v3(reps=reps, **kw)
        elif variant in ("uw16", "uw8"):
            nc = build_program_v2(reps=reps, st8=(variant == "uw8"), **kw)
        else:
            nc = build_program(variant=variant, reps=reps, **kw)
        _CACHE[key] = Runner(nc, n_inner=n_inner)
    return _CACHE[key]


def v4_feasible_S(molecule_id, num_molecules, ncores=NCORES):
    """Largest S from the candidate list so every core can fill all P rows
    with single-class chunks of S atoms (see host_prep_v4)."""
    ids = np.asarray(molecule_id).astype(np.int64)
    counts = np.bincount(ids, minlength=int(num_molecules))
    c_atom = counts[ids]
    OWN = ids.shape[0] // ncores
    for S in (V4_S, 1470, 1280, 1080, 880):
        ok = True
        for c in range(ncores):
            _, pops = np.unique(c_atom[c * OWN:(c + 1) * OWN],
                                return_counts=True)
            if (pops // S).sum() < P:
                ok = False
                break
        if ok:
            return S
    raise ValueError("no feasible v4 row-class packing")


def kernel(states_x, x_target, molecule_id, num_molecules):
    if DEFAULT_VARIANT == "v4":
        S = v4_feasible_S(molecule_id, num_molecules)
        runner = get_runner("v4", **({} if S == V4_S else {"S": S}))
        in_maps, V = host_prep_v4(states_x, x_target, molecule_id,
                                  num_molecules, S=S)
        results = runner.run(in_maps)
        nsl = len(v4_segs(V4_X1, V4_X2, R2_FULL * 3))
        return combine_v4(results, V, nsl=nsl)
    runner = get_runner(DEFAULT_VARIANT)
    in_maps, V = host_prep(states_x, x_target, molecule_id, num_molecules,
                           variant=DEFAULT_VARIANT)
    results = runner.run(in_maps)
    nch = 1 if DEFAULT_VARIANT == "v3" else NCHUNKS2_FULL
    return combine(results, V, nchunks=nch)

